# revision 3
# baseline (speedup 1.0000x reference)
"""Trainium2 Bass kernel for nn_DeepSeekBlock — fp8 DoubleRow version.

Sharding: data-parallel over (batch, query-half): core c handles batch c//2,
query half c%2. Per-core inputs are COLUMN-ROTATED so the core's query half
always occupies columns 0..511 — keeping the SPMD program uniform while the
causal mask / rope tables / key order are per-core data.

All large matmuls run in fp8e4 (e4m3) with perf_mode=DoubleRow: lhsT/rhs
carry two 128-deep k-tiles stacked in a middle AP dim ([K,2,M] x [K,2,N]).
Weights are host-scaled x32 (rope-q x8, shared-down x8) and descaled via
ACT-copy scales; activations carry explicit power-of-2 scales (see comments).
Causal masking is folded into the scores PSUM via a 64*I "maskid" DoubleRow
matmul against per-core {0,-240} mask data, so exp can write fp8 probs
directly and attn@V stays DoubleRow. Router/top-2 runs in fp32 off the
pre-rstd LN2 residuals; MoE uses sparse dispatch (capacity 192/expert) with
the 7 routed scatters + shared-expert down accumulated in one PSUM pass.
"""
import sys
for _p in ('/opt/trn_rl_repo', '/opt/pypackages'):
    if _p not in sys.path:
        sys.path.insert(0, _p)

import numpy as np
import ml_dtypes

import bass_rust
import concourse.bass as bass
import concourse.mybir as mybir
import concourse.tile as tile
from concourse.bass_utils import run_bass_kernel_spmd
from concourse.vector_clock import ScopedClock
from contextlib import ExitStack

# ---------------------------------------------------------------------------
# Patch Tile for this toolchain's 1-sync-wait-per-instruction codegen limit
# (same fix as the bf16 baseline).
# ---------------------------------------------------------------------------
_MAX_WAITS = 1
_orig_tile_add = tile.TileContext._add_instruction


def _split_waits(tc, inst):
    si = inst.sync_info
    if not si or not si.on_wait or len(si.on_wait) <= _MAX_WAITS:
        return
    waits = list(si.on_wait)
    keep, extra = waits[-_MAX_WAITS:], waits[:-_MAX_WAITS]
    eng = tc.nc.engines[inst.engine]
    for w in extra:
        nop = eng.nop(nofuse=True, hint="waitfix")
        nop.ins.sync_info = bass_rust.SyncInfo(on_wait=[w], on_update=[])
    inst.sync_info = bass_rust.SyncInfo(
        on_wait=keep, on_update=list(si.on_update) if si.on_update else [])


def _patched_tile_add(self, inst):
    if inst.engine != mybir.EngineType.Unassigned:
        _split_waits(self, inst)
    _orig_tile_add(self, inst)


def _patched_drain_and_barrier(self, tick_clock, wait_clock):
    probe = self.nc.sync.nop(nofuse=True, hint="waitfix_tail")
    wait_clock.add_sem_waits(
        probe.ins, ScopedClock({None: tick_clock.global_clock}))
    _split_waits(self, probe.ins)
    self.nc.sync.drain()
    self.nc.all_engine_barrier()
    assert self.sems is not None
    popped = self.nc._tile_sem_poison_stack.pop()
    assert popped is self._sem_poison
    self.nc.clear_and_free_semaphores(list(self.sems.allocated().values()))
    self.nc.all_engine_barrier()


if not getattr(tile.TileContext, "_waitfix_installed", False):
    tile.TileContext._add_instruction = _patched_tile_add
    tile.TileContext._drain_and_barrier = _patched_drain_and_barrier
    tile.TileContext._waitfix_installed = True


F32 = mybir.dt.float32
BF16 = mybir.dt.bfloat16
F8 = mybir.dt.float8e4
F83 = mybir.dt.float8e3
AX = mybir.AxisListType
ALU = mybir.AluOpType
ACTF = mybir.ActivationFunctionType
DR = mybir.MatmulPerfMode.DoubleRow

H = 1024; NH = 16; HD = 64; RD = 32; L = 256
E = 8; ER = 7; I = 2048
B = 4; T = 1024; TQ = 512
BASE = 10000.0; EPS = 1e-5
C = 192          # routed-expert capacity per core
WS = 32.0        # weight scale for fp8

_BF = ml_dtypes.bfloat16
_F8 = ml_dtypes.float8_e4m3
_F83 = ml_dtypes.float8_e3m4


def _f8(x, scale=1.0):
    a = np.asarray(x, np.float32) * scale
    return np.clip(a, -240.0, 240.0).astype(_F8)


def _bf(x, scale=1.0):
    return np.ascontiguousarray(np.asarray(x, np.float32) * scale).astype(_BF)


def _f83(x, scale=1.0):
    a = np.asarray(x, np.float32) * scale
    return np.clip(a, -15.5, 15.5).astype(_F83)


def host_prep(inputs):
    ln1 = np.asarray(inputs['ln1_w'], np.float32)
    ln2 = np.asarray(inputs['ln2_w'], np.float32)
    w = {}
    w['kv_dT'] = _f8(np.asarray(inputs['kv_d']).T * ln1[:, None], WS)     # [H,L]
    w['q_dT'] = _f8(np.asarray(inputs['q_d']).T * ln1[:, None], WS)
    k_uT = np.asarray(inputs['k_u'], np.float32).T                        # [L,NH*HD]
    q_uT = np.asarray(inputs['q_u'], np.float32).T
    nope = np.concatenate([np.arange(h * HD + RD, (h + 1) * HD) for h in range(NH)])
    w['k_uT_nope'] = _f8(k_uT[:, nope], WS)                               # [L,512]
    w['q_uT_nope'] = _f8(q_uT[:, nope], WS)
    rkT = np.asarray(inputs['rope_k_w'], np.float32).T * ln1[:, None]     # [H,512]
    rqT = np.asarray(inputs['rope_q_w'], np.float32).T                    # [L,512]

    def rot_cols(wt):
        out = np.empty_like(wt)
        for h in range(NH):
            c = h * RD
            out[:, c:c + RD // 2] = -wt[:, c + RD // 2:c + RD]
            out[:, c + RD // 2:c + RD] = wt[:, c:c + RD // 2]
        return out

    w['rkT_a'] = _f8(rkT, WS); w['rkT_b'] = _f8(rot_cols(rkT), WS)
    w['rqT_a'] = _f8(rqT, 8.0); w['rqT_b'] = _f8(rot_cols(rqT), 8.0)
    v_uT = np.asarray(inputs['v_u'], np.float32).T                        # [L,NH*HD]
    vpad = np.zeros((L, NH * 80), np.float32)
    for h in range(NH):
        vpad[:, h * 80:h * 80 + HD] = v_uT[:, h * HD:(h + 1) * HD]
    w['v_uT_pad'] = _f8(vpad, WS)                                         # [L,1280]
    owT32 = np.asarray(inputs['o_w'], np.float32).T * WS                  # [H,H]
    owhi = _f8(owT32)
    w['o_wT_hi'] = owhi
    w['o_wT_lo'] = _f8(owT32 - owhi.astype(np.float32))
    rwT = np.asarray(inputs['router_w'], np.float32).T * ln2[:, None]     # [H,7]
    w['router_wT_pad'] = np.concatenate(
        [rwT, np.zeros((H, 1), np.float32)], 1).astype(np.float32)        # [H,8]
    bias = np.asarray(inputs['routing_bias'], np.float32)
    bias_pad = np.concatenate([bias, np.full((1,), -30.0, np.float32)])
    w['bias_tile'] = np.broadcast_to(bias_pad, (128, 8)).astype(np.float32).copy()
    gs, us, dhs, dls = [], [], [], []
    for e in range(ER):
        gs.append(_f8(np.asarray(inputs['routed_gate'][e]).T * ln2[:, None], WS))
        us.append(_f8(np.asarray(inputs['routed_up'][e]).T * ln2[:, None], WS))
        d32 = np.asarray(inputs['routed_down'][e], np.float32).T * WS
        dhi = _f8(d32)
        dhs.append(dhi)
        dls.append(_f8(d32 - dhi.astype(np.float32)))
    dsh32 = np.asarray(inputs['shared_down'][0], np.float32).T * 8.0  # shared x8
    dshhi = _f8(dsh32)
    dhs.append(dshhi)
    dls.append(_f8(dsh32 - dshhi.astype(np.float32)))
    w['gate_wT'] = np.stack(gs)      # [7,H,I] f8
    w['up_wT'] = np.stack(us)        # [7,H,I] f8
    w['down_hi'] = np.stack(dhs)     # [8,I,H] f8
    w['down_lo'] = np.stack(dls)     # [8,I,H] f8
    w['gate_sh'] = _f8(np.asarray(inputs['shared_gate'][0]).T * ln2[:, None], WS)
    w['up_sh'] = _f8(np.asarray(inputs['shared_up'][0]).T * ln2[:, None], WS)

    w['identity'] = np.eye(128, dtype=np.float32)
    w['ones_col'] = _bf(np.ones((128, 1)))
    w['ones_row'] = _bf(np.ones((1, 128)))
    w['ones_one'] = np.ones((1, 1), np.float32)
    onesc2 = np.zeros((128, 32), np.float32)
    onesc2[:, 0] = 1.0; onesc2[:, 16] = 1.0
    w['onesc2'] = _f8(onesc2)
    kk, mm_ = np.meshgrid(np.arange(128), np.arange(128), indexing='ij')
    w['tri'] = _bf((kk <= mm_).astype(np.float32))
    w['iota192'] = np.broadcast_to(np.arange(192, dtype=np.float32), (128, 192)).copy()
    w['iota96_0'] = np.arange(96, dtype=np.float32).reshape(96, 1).copy()
    w['iota96_1'] = (96.0 + np.arange(96, dtype=np.float32)).reshape(96, 1).copy()
    rs = np.zeros((8, ER * 128), np.float32)
    for e in range(ER):
        rs[e, e * 128:(e + 1) * 128] = 1.0
    w['rowsel'] = _bf(rs)
    mid = np.zeros((64, 2, 128), np.float32)
    for p in range(64):
        mid[p, 0, p] = 64.0
        mid[p, 1, p + 64] = 64.0
    w['maskid'] = _f8(mid.reshape(64, 256))

    # rope tables: 32-row pattern tiled to 128 rows, scaled 1/4 (see scales)
    pos = np.arange(T, dtype=np.float32)
    inv = 1.0 / (BASE ** (np.arange(0, RD, 2, dtype=np.float32) / RD))
    emb = np.concatenate([pos[:, None] * inv[None, :]] * 2, 1)            # [T,32]
    cosk = np.tile(np.cos(emb).T.astype(np.float32), (4, 1)) * 0.25       # [128,T]
    sink = np.tile(np.sin(emb).T.astype(np.float32), (4, 1)) * 0.25

    x = np.asarray(inputs['x'], np.float32)                               # [B,T,H]
    cores = []
    for c in range(8):
        b, qh = c // 2, c % 2
        # column rotation: core's query half first
        perm = np.concatenate([np.arange(qh * TQ, (qh + 1) * TQ),
                               np.arange((1 - qh) * TQ, (2 - qh) * TQ)])
        xT = np.ascontiguousarray(x[b].T[:, perm])                        # [H,T]
        d = {}
        d['xT_rot'] = _bf(xT)
        d['xT_rot8'] = _f8(xT)
        d['cos_k'] = np.ascontiguousarray(cosk[:, perm])
        d['sin_k'] = np.ascontiguousarray(sink[:, perm])
        korig = perm                                  # rotated key -> original
        qorig = qh * TQ + np.arange(TQ)               # query col -> original
        Mfull = np.where(korig[:, None] <= qorig[None, :], 0.0, -240.0)   # [T,TQ]
        m8 = np.zeros((64, 16, TQ), np.float32)
        for kt in range(8):
            m8[:, 2 * kt, :] = Mfull[kt * 128:kt * 128 + 64, :]
            m8[:, 2 * kt + 1, :] = Mfull[kt * 128 + 64:(kt + 1) * 128, :]
        d['mask8'] = _f8(m8.reshape(64, 16 * TQ))
        cores.append(d)
    return w, cores


def build():
    nc = bass.Bass("TRN2", target_bir_lowering=False, debug=False)

    def din(name, shape, dt):
        return nc.dram_tensor(name, list(shape), dt, kind="ExternalInput").ap()

    kv_dT = din("kv_dT", (H, L), F8)
    q_dT = din("q_dT", (H, L), F8)
    k_uT_nope = din("k_uT_nope", (L, 512), F8)
    q_uT_nope = din("q_uT_nope", (L, 512), F8)
    rkT_a = din("rkT_a", (H, 512), F8)
    rkT_b = din("rkT_b", (H, 512), F8)
    rqT_a = din("rqT_a", (L, 512), F8)
    rqT_b = din("rqT_b", (L, 512), F8)
    v_uT_pad = din("v_uT_pad", (L, 1280), F8)
    o_wT_hi = din("o_wT_hi", (H, H), F8)
    o_wT_lo = din("o_wT_lo", (H, H), F8)
    router_wT_pad = din("router_wT_pad", (H, 8), F32)
    bias_tile_d = din("bias_tile", (128, 8), F32)
    gate_wT = din("gate_wT", (ER, H, I), F8)
    up_wT = din("up_wT", (ER, H, I), F8)
    down_hi = din("down_hi", (E, I, H), F8)
    down_lo = din("down_lo", (E, I, H), F8)
    gate_sh = din("gate_sh", (H, I), F8)
    up_sh = din("up_sh", (H, I), F8)
    identity_d = din("identity", (128, 128), F32)
    ones_col_d = din("ones_col", (128, 1), BF16)
    ones_row_d = din("ones_row", (1, 128), BF16)
    ones_one_d = din("ones_one", (1, 1), F32)
    onesc2_d = din("onesc2", (128, 32), F8)
    tri_d = din("tri", (128, 128), BF16)
    iota192_d = din("iota192", (128, 192), F32)
    iota96_0_d = din("iota96_0", (96, 1), F32)
    iota96_1_d = din("iota96_1", (96, 1), F32)
    rowsel_d = din("rowsel", (8, ER * 128), BF16)
    maskid_d = din("maskid", (64, 256), F8)
    # per-core
    xT_rot = din("xT_rot", (H, T), BF16)
    xT_rot8 = din("xT_rot8", (H, T), F8)
    cos_k = din("cos_k", (128, T), F32)
    sin_k = din("sin_k", (128, T), F32)
    mask_d = din("mask8", (64, 16 * TQ), F8)

    outT = nc.dram_tensor("outT", [H, TQ], F32, kind="ExternalOutput").ap()

    EXPS = 0.125 / 64.0   # exp scale: 1/sqrt(HD) / (8k * 8q psum scale)

    def blk3(dram_ap, nk):
        return dram_ap.rearrange("(k p) c -> p k c", p=128)

    with tile.TileContext(nc, pool_alloc_mode="queue") as tc, ExitStack() as ctx:
        pp = ctx.enter_context(tc.tile_pool(name="persist", bufs=1))

        # persistent small tiles
        ident = pp.tile([128, 128], F32, tag="ident", name="ident")
        onesc = pp.tile([128, 1], BF16, tag="onesc", name="onesc")
        onesr = pp.tile([1, 128], BF16, tag="onesr", name="onesr")
        onesone = pp.tile([1, 1], F32, tag="onesone", name="onesone")
        onesc2 = pp.tile([128, 2, 16], F8, tag="onesc2", name="onesc2")
        trit = pp.tile([128, 128], BF16, tag="trit", name="trit")
        iota192 = pp.tile([128, 192], F32, tag="iota192", name="iota192")
        iota96 = [pp.tile([96, 1], F32, tag=f"iota96_{i}", name=f"iota96_{i}")
                  for i in range(2)]
        rowselt = pp.tile([8, ER * 128], BF16, tag="rowselt", name="rowselt")
        biast = pp.tile([128, 8], F32, tag="biast", name="biast")
        maskid = pp.tile([64, 2, 128], F8, tag="maskid", name="maskid")

        # tiles that span phases A..B, released to free SBUF for MoE streaming
        pab = tc.alloc_tile_pool(name="spanAB", bufs=1)
        mask8t = pab.tile([64, 16, TQ], F8, tag="mask8t", name="mask8t")

        nc.sync.dma_start(ident[:], identity_d[:])
        nc.sync.dma_start(onesc[:], ones_col_d[:])
        nc.sync.dma_start(onesr[:], ones_row_d[:])
        nc.sync.dma_start(onesone[:], ones_one_d[:])
        nc.sync.dma_start(onesc2.rearrange("p a b -> p (a b)")[:], onesc2_d[:])
        nc.sync.dma_start(trit[:], tri_d[:])
        nc.sync.dma_start(iota192[:], iota192_d[:])
        nc.sync.dma_start(iota96[0][:], iota96_0_d[:])
        nc.sync.dma_start(iota96[1][:], iota96_1_d[:])
        nc.sync.dma_start(rowselt[:], rowsel_d[:])
        nc.sync.dma_start(biast[:], bias_tile_d[:])
        nc.sync.dma_start(maskid.rearrange("p a b -> p (a b)")[:], maskid_d[:])
        nc.sync.dma_start(mask8t.rearrange("p a b -> p (a b)")[:], mask_d[:])

        # persistent activations
        xbs = pp.tile([128, 8, T], BF16, tag="xbs", name="xbs")
        xbs8_early = [pp.tile([128, 8, 512], F8, tag=f"xbs8_{i}", name=f"xbs8_{i}")
                      for i in range(2)]
        nc.sync.dma_start(xbs8_early[0][:], blk3(xT_rot8, 8)[:, :, 0:512])
        nc.sync.dma_start(xbs8_early[1][:], blk3(xT_rot8, 8)[:, :, 512:1024])
        nc.sync.dma_start(xbs[:], blk3(xT_rot, 8))
        kh4 = [pab.tile([128, 2, T], F8, tag=f"kh4_{i}", name=f"kh4_{i}") for i in range(4)]
        qh4 = [pab.tile([128, 2, TQ], F8, tag=f"qh4_{i}", name=f"qh4_{i}") for i in range(4)]
        vpair = [pab.tile([128, 2, 1280], F8, tag=f"vp{i}", name=f"vp{i}") for i in range(4)]
        yT8 = pp.tile([128, 8, TQ], F8, tag="yT8", name="yT8")
        x2T = [pp.tile([128, TQ], F32, tag=f"x2T{i}", name=f"x2T{i}") for i in range(8)]
        xn2b8 = pp.tile([128, 8, TQ], F8, tag="xn2b8", name="xn2b8")
        xn2tokP = pp.tile([128, 4, H], F8, tag="xn2tokP", name="xn2tokP")
        rstdcol = [pp.tile([128, 1], F32, tag=f"rsc{i}", name=f"rsc{i}") for i in range(4)]
        wb = [pp.tile([128, TQ], BF16, tag=f"wb{i}", name=f"wb{i}") for i in range(ER)]
        posm = [pp.tile([128, 8], F32, tag=f"posm{i}", name=f"posm{i}") for i in range(4)]
        w8all = pp.tile([8, TQ], BF16, tag="w8all", name="w8all")
        pos8all = pp.tile([8, TQ], BF16, tag="pos8all", name="pos8all")


        # ---------- phase A: ln1 + latents + k/q/v build ----------
        with tc.tile_pool(name="phA", bufs=1) as pa, \
             tc.tile_pool(name="phA_ps", bufs=1, space="PSUM") as pza:
            xbs8 = xbs8_early
            ck = pa.tile([128, T], F32, tag="ck", name="ck")
            sk = pa.tile([128, T], F32, tag="sk", name="sk")
            nc.sync.dma_start(ck[:], cos_k[:])
            nc.sync.dma_start(sk[:], sin_k[:])

            xnb8 = [pa.tile([128, 8, 512], F8, tag=f"xnb8_{i}", name=f"xnb8_{i}")
                    for i in range(2)]
            # --- LN1 over full T, feature-major, stats via fp8 DR matmuls ---
            for ch in range(2):
                cs = bass.ds(ch * 512, 512)
                ps_s = pza.tile([1, 512], F32, tag="ps_s", name="ps_s", bufs=1)
                for p2 in range(4):
                    nc.tensor.matmul(ps_s[:], onesc2[:, :, 0:1],
                                     xbs8[ch][:, 2 * p2:2 * p2 + 2, :],
                                     start=(p2 == 0), stop=(p2 == 3), perf_mode=DR)
                sq8 = pa.tile([128, 8, 512], F8, tag="sq8", name="sq8", bufs=2)
                for g in range(4):
                    eng = nc.vector if ch == 0 else nc.gpsimd
                    eng.tensor_tensor(
                        sq8[:, 2 * g:2 * g + 2, :],
                        xbs8[ch][:, 2 * g:2 * g + 2, :],
                        xbs8[ch][:, 2 * g:2 * g + 2, :], op=ALU.mult)
                ps_q = pza.tile([1, 512], F32, tag="ps_q", name="ps_q", bufs=1)
                for p2 in range(4):
                    nc.tensor.matmul(ps_q[:], onesc2[:, :, 0:1],
                                     sq8[:, 2 * p2:2 * p2 + 2, :],
                                     start=(p2 == 0), stop=(p2 == 3), perf_mode=DR)
                mu16 = pa.tile([1, 512], BF16, tag="mu16", name="mu16", bufs=1)
                nc.vector.tensor_scalar(mu16[:], ps_s[:], 1.0 / H, None, op0=ALU.mult)
                ex2 = pa.tile([1, 512], F32, tag="ex2", name="ex2", bufs=1)
                nc.vector.tensor_scalar(ex2[:], ps_q[:], 1.0 / H, None, op0=ALU.mult)
                musq = pa.tile([1, 512], F32, tag="musq", name="musq", bufs=1)
                nc.vector.tensor_tensor(musq[:], mu16[:], mu16[:], op=ALU.mult)
                var = pa.tile([1, 512], F32, tag="var", name="var", bufs=1)
                nc.vector.tensor_sub(var[:], ex2[:], musq[:])
                nc.vector.tensor_scalar_add(var[:], var[:], EPS)
                sd = pa.tile([1, 512], F32, tag="sd", name="sd", bufs=1)
                nc.scalar.activation(sd[:], var[:], ACTF.Sqrt)
                rstd16 = pa.tile([1, 512], BF16, tag="rstd16", name="rstd16", bufs=1)
                with nc.allow_low_precision(reason="ln1 rstd bf16 bcast"):
                    nc.vector.reciprocal(rstd16[:], sd[:])
                pm_mu = pza.tile([128, 512], F32, tag="bc_mu", name="bc_mu", bufs=1)
                nc.tensor.matmul(pm_mu[:], onesr[:], mu16[:], start=True, stop=True)
                pm_rs = pza.tile([128, 512], F32, tag="bc_rs", name="bc_rs", bufs=1)
                nc.tensor.matmul(pm_rs[:], onesr[:], rstd16[:], start=True, stop=True)
                mub = pa.tile([128, 512], BF16, tag="mub", name="mub", bufs=2)
                nc.scalar.activation(mub[:], pm_mu[:], ACTF.Copy)
                rsb = pa.tile([128, 512], BF16, tag="rsb", name="rsb", bufs=2)
                nc.scalar.activation(rsb[:], pm_rs[:], ACTF.Copy)
                for kt in range(8):
                    t_ = pa.tile([128, 512], BF16, tag="lnt", name="lnt", bufs=3)
                    nc.vector.tensor_sub(t_[:], xbs[:, kt, cs], mub[:])
                    nc.vector.tensor_tensor(xnb8[ch][:, kt, :], t_[:], rsb[:],
                                            op=ALU.mult)

            # --- latents ---
            kvd = pa.tile([128, 8, L], F8, tag="kvd", name="kvd")
            nc.sync.dma_start(kvd[:],
                              blk3(kv_dT, 8))
            qd = pa.tile([128, 8, L], F8, tag="qd", name="qd")
            nc.sync.dma_start(qd[:],
                              blk3(q_dT, 8))
            kvlat8 = [pa.tile([128, 2, 512], F8, tag=f"kvlat8_{i}", name=f"kvlat8_{i}")
                      for i in range(2)]
            qlat8 = pa.tile([128, 2, TQ], F8, tag="qlat8", name="qlat8")
            for mt in range(2):
                for ch in range(2):
                    cs = bass.ds(ch * 512, 512)
                    pm = pza.tile([128, 512], F32, tag="mm", name="mm", bufs=2)
                    for p2 in range(4):
                        nc.tensor.matmul(pm[:], kvd[:, 2 * p2:2 * p2 + 2,
                                                    mt * 128:(mt + 1) * 128],
                                         xnb8[ch][:, 2 * p2:2 * p2 + 2, :],
                                         start=(p2 == 0), stop=(p2 == 3), perf_mode=DR)
                    nc.scalar.activation(kvlat8[ch][:, mt, :], pm[:], ACTF.Copy,
                                         scale=1.0 / 8)
                pm = pza.tile([128, 512], F32, tag="mm", name="mm", bufs=2)
                for p2 in range(4):
                    nc.tensor.matmul(pm[:], qd[:, 2 * p2:2 * p2 + 2,
                                               mt * 128:(mt + 1) * 128],
                                     xnb8[0][:, 2 * p2:2 * p2 + 2, :],
                                     start=(p2 == 0), stop=(p2 == 3), perf_mode=DR)
                nc.scalar.activation(qlat8[:, mt, :], pm[:], ACTF.Copy, scale=1.0 / 8)

            # --- k/q nope -> kh4/qh4 slot 1 ---
            kun = pa.tile([128, 2, 512], F8, tag="kun", name="kun")
            nc.sync.dma_start(kun[:],
                              blk3(k_uT_nope, 2))
            qun = pa.tile([128, 2, 512], F8, tag="qun", name="qun")
            nc.sync.dma_start(qun[:],
                              blk3(q_uT_nope, 2))
            for mt in range(4):
                for ch in range(2):
                    cs = bass.ds(ch * 512, 512)
                    pm = pza.tile([128, 512], F32, tag="mm", name="mm", bufs=2)
                    nc.tensor.matmul(pm[:], kun[:, :, mt * 128:(mt + 1) * 128],
                                     kvlat8[ch][:], start=True, stop=True,
                                     perf_mode=DR)
                    nc.scalar.activation(kh4[mt][:, 1, cs], pm[:], ACTF.Copy,
                                         scale=1.0 / 16)
                pm = pza.tile([128, 512], F32, tag="mm", name="mm", bufs=2)
                nc.tensor.matmul(pm[:], qun[:, :, mt * 128:(mt + 1) * 128],
                                 qlat8[:], start=True, stop=True, perf_mode=DR)
                nc.scalar.activation(qh4[mt][:, 1, :], pm[:], ACTF.Copy,
                                     scale=1.0 / 16)

            # --- rope -> kh4/qh4 slot 0  (8*raw = pma*ck + pmb*sk) ---
            rka = pa.tile([128, 8, 512], F8, tag="rka", name="rka")
            nc.sync.dma_start(rka[:],
                              blk3(rkT_a, 8))
            rkb = pa.tile([128, 8, 512], F8, tag="rkb", name="rkb")
            nc.sync.dma_start(rkb[:],
                              blk3(rkT_b, 8))
            rqa = pa.tile([128, 2, 512], F8, tag="rqa", name="rqa")
            nc.sync.dma_start(rqa[:],
                              blk3(rqT_a, 2))
            rqb = pa.tile([128, 2, 512], F8, tag="rqb", name="rqb")
            nc.sync.dma_start(rqb[:],
                              blk3(rqT_b, 2))
            for mt in range(4):
                for ch in range(2):
                    cs = bass.ds(ch * 512, 512)
                    pma = pza.tile([128, 512], F32, tag="mm", name="mm", bufs=2)
                    pmb = pza.tile([128, 512], F32, tag="mm2", name="mm2", bufs=2)
                    for p2 in range(4):
                        nc.tensor.matmul(pma[:], rka[:, 2 * p2:2 * p2 + 2,
                                                     mt * 128:(mt + 1) * 128],
                                         xnb8[ch][:, 2 * p2:2 * p2 + 2, :],
                                         start=(p2 == 0), stop=(p2 == 3), perf_mode=DR)
                        nc.tensor.matmul(pmb[:], rkb[:, 2 * p2:2 * p2 + 2,
                                                     mt * 128:(mt + 1) * 128],
                                         xnb8[ch][:, 2 * p2:2 * p2 + 2, :],
                                         start=(p2 == 0), stop=(p2 == 3), perf_mode=DR)
                    t1 = pa.tile([128, 512], F32, tag="rt1", name="rt1", bufs=2)
                    nc.vector.tensor_tensor(t1[:], pma[:], ck[:, cs], op=ALU.mult)
                    t2 = pa.tile([128, 512], F32, tag="rt2", name="rt2", bufs=2)
                    nc.vector.tensor_tensor(t2[:], pmb[:], sk[:, cs], op=ALU.mult)
                    nc.gpsimd.tensor_add(kh4[mt][:, 0, cs], t1[:], t2[:])
                pma = pza.tile([128, 512], F32, tag="mm", name="mm", bufs=2)
                pmb = pza.tile([128, 512], F32, tag="mm2", name="mm2", bufs=2)
                nc.tensor.matmul(pma[:], rqa[:, :, mt * 128:(mt + 1) * 128],
                                 qlat8[:], start=True, stop=True, perf_mode=DR)
                nc.tensor.matmul(pmb[:], rqb[:, :, mt * 128:(mt + 1) * 128],
                                 qlat8[:], start=True, stop=True, perf_mode=DR)
                t1 = pa.tile([128, 512], F32, tag="rt1", name="rt1", bufs=2)
                nc.vector.tensor_tensor(t1[:], pma[:], ck[:, 0:512], op=ALU.mult)
                t2 = pa.tile([128, 512], F32, tag="rt2", name="rt2", bufs=2)
                nc.vector.tensor_tensor(t2[:], pmb[:], sk[:, 0:512], op=ALU.mult)
                nc.gpsimd.tensor_add(qh4[mt][:, 0, :], t1[:], t2[:])

            # --- v (token-major, 80-padded, ones col) ---
            vup = pa.tile([128, 2, 1280], F8, tag="vup", name="vup")
            nc.sync.dma_start(vup[:],
                              blk3(v_uT_pad, 2))
            for tm in range(8):
                vflat = vpair[tm // 2].rearrange("p a b -> p (a b)")
                half = tm % 2
                for n0, nn in ((0, 512), (512, 512), (1024, 256)):
                    pm = pza.tile([128, 512], F32, tag="mm", name="mm", bufs=2)
                    nc.tensor.matmul(pm[:, 0:nn],
                                     kvlat8[tm // 4][:, :,
                                                     (tm % 4) * 128:(tm % 4 + 1) * 128],
                                     vup[:, :, n0:n0 + nn],
                                     start=True, stop=True, perf_mode=DR)
                    nc.scalar.activation(vflat[:, half * 1280 + n0:half * 1280 + n0 + nn],
                                         pm[:, 0:nn], ACTF.Copy, scale=1.0 / 16)
                v4 = vpair[tm // 2].rearrange("p a (h c) -> p a h c", c=80)
                nc.gpsimd.memset(v4[:, half, :, 64:65], 1.0)

        # ---------- phase B: attention ----------
        with tc.tile_pool(name="phB", bufs=1) as pb, \
             tc.tile_pool(name="phB_ps", bufs=1, space="PSUM") as pzb:
            GROUPS = [(0, 3), (3, 3), (6, 2)]
            for h in range(NH):
                src, off = h // 4, (h % 4) * 32
                p8 = pb.tile([128, 8, TQ], BF16, tag="p8", name="p8", bufs=2)
                for (k0, klen) in GROUPS:
                    ssg = pzb.tile([128, 1536], F32, tag="ssg", name="ssg", bufs=2)
                    for i in range(klen):
                        kt = k0 + i
                        ss = ssg[:, i * 512:(i + 1) * 512]
                        nc.tensor.matmul(ss, kh4[src][off:off + 32, :,
                                                      kt * 128:(kt + 1) * 128],
                                         qh4[src][off:off + 32, :, :],
                                         start=True, stop=False, perf_mode=DR,
                                         tile_position=(off, 0))
                        nc.tensor.matmul(ss, maskid[:],
                                         mask8t[:, 2 * kt:2 * kt + 2, :],
                                         start=False, stop=True, perf_mode=DR)
                    nc.scalar.activation(
                        p8[:, k0:k0 + klen, :].rearrange("p a b -> p (a b)"),
                        ssg[:, 0:klen * 512], ACTF.Exp, scale=EXPS)
                py = pzb.tile([65, TQ], F32, tag="py", name="py", bufs=2)
                for s in range(8):
                    nc.tensor.matmul(py[:], vpair[s // 2][:, s % 2,
                                                          80 * h:80 * h + 65],
                                     p8[:, s, :],
                                     start=(s == 0), stop=(s == 7))
                r1 = pb.tile([1, TQ], BF16, tag="r1", name="r1", bufs=2)
                with nc.allow_low_precision(reason="softmax recip bf16 bcast"):
                    nc.vector.reciprocal(r1[:], py[64:65, :])
                prb = pzb.tile([65, TQ], F32, tag="py", name="prb", bufs=2)
                nc.tensor.matmul(prb[0:64, :], onesr[:, 0:64], r1[:],
                                 start=True, stop=True)
                rbs = pb.tile([64, TQ], BF16, tag="rbs", name="rbs", bufs=2)
                nc.vector.tensor_copy(rbs[:], prb[0:64, :])
                nc.vector.tensor_tensor(
                    yT8[(h % 2) * 64:(h % 2) * 64 + 64, h // 2, :],
                    py[0:64, :], rbs[:], op=ALU.mult)

        pab.release()

        # expert-weight streaming pool: opened before C so MoE weights
        # prefetch across phase C while its engines run
        pstr = tc.alloc_tile_pool(name="phD_stream", bufs=1)

        # ---------- phase C: o_proj + residual + ln2 + router + top-2 ----------
        with tc.tile_pool(name="phC", bufs=1) as pc:
          tln = [pc.tile([128, TQ], F32, tag=f"tln{i}", name=f"tln{i}")
                 for i in range(8)]
          with tc.tile_pool(name="phC_ps1", bufs=1, space="PSUM") as pzc:
            owh = pc.tile([128, 8, H], F8, tag="owh", name="owh")
            nc.sync.dma_start(owh[:], blk3(o_wT_hi, 8))
            owl = pc.tile([128, 8, H], F8, tag="owl", name="owl")
            nc.sync.dma_start(owl[:], blk3(o_wT_lo, 8))
            for mt in range(8):
                pm = pzc.tile([128, TQ], F32, tag="mm", name="mm", bufs=2)
                for p2 in range(4):
                    nc.tensor.matmul(pm[:], owh[:, 2 * p2:2 * p2 + 2,
                                                mt * 128:(mt + 1) * 128],
                                     yT8[:, 2 * p2:2 * p2 + 2, :],
                                     start=(p2 == 0), stop=False, perf_mode=DR)
                for p2 in range(4):
                    nc.tensor.matmul(pm[:], owl[:, 2 * p2:2 * p2 + 2,
                                                mt * 128:(mt + 1) * 128],
                                     yT8[:, 2 * p2:2 * p2 + 2, :],
                                     start=False, stop=(p2 == 3), perf_mode=DR)
                nc.vector.scalar_tensor_tensor(x2T[mt][:], pm[:], 1.0 / 256,
                                               xbs[:, mt, 0:512],
                                               op0=ALU.mult, op1=ALU.add)

            # --- ln2 stats (fp8 DR) ---
            x2s8 = pc.tile([128, 8, TQ], F8, tag="x2s8", name="x2s8")
            for mt in range(8):
                nc.scalar.activation(x2s8[:, mt, :], x2T[mt][:], ACTF.Copy)
            ps_s = pzc.tile([1, 512], F32, tag="ps_s", name="ps_s", bufs=1)
            for p2 in range(4):
                nc.tensor.matmul(ps_s[:], onesc2[:, :, 0:1],
                                 x2s8[:, 2 * p2:2 * p2 + 2, :],
                                 start=(p2 == 0), stop=(p2 == 3), perf_mode=DR)
            sq8c = pc.tile([128, 8, TQ], F8, tag="sq8c", name="sq8c")
            for g in range(4):
                eng = nc.vector if g % 2 == 0 else nc.gpsimd
                eng.tensor_tensor(sq8c[:, 2 * g:2 * g + 2, :],
                                  x2s8[:, 2 * g:2 * g + 2, :],
                                  x2s8[:, 2 * g:2 * g + 2, :], op=ALU.mult)
            ps_q = pzc.tile([1, 512], F32, tag="ps_q", name="ps_q", bufs=1)
            for p2 in range(4):
                nc.tensor.matmul(ps_q[:], onesc2[:, :, 0:1],
                                 sq8c[:, 2 * p2:2 * p2 + 2, :],
                                 start=(p2 == 0), stop=(p2 == 3), perf_mode=DR)
            mu16 = pc.tile([1, 512], BF16, tag="mu16c", name="mu16c")
            nc.vector.tensor_scalar(mu16[:], ps_s[:], 1.0 / H, None, op0=ALU.mult)
            ex2 = pc.tile([1, 512], F32, tag="ex2c", name="ex2c")
            nc.vector.tensor_scalar(ex2[:], ps_q[:], 1.0 / H, None, op0=ALU.mult)
            musq = pc.tile([1, 512], F32, tag="musqc", name="musqc")
            nc.vector.tensor_tensor(musq[:], mu16[:], mu16[:], op=ALU.mult)
            var = pc.tile([1, 512], F32, tag="varc", name="varc")
            nc.vector.tensor_sub(var[:], ex2[:], musq[:])
            nc.vector.tensor_scalar_add(var[:], var[:], EPS)
            sd = pc.tile([1, 512], F32, tag="sdc", name="sdc")
            nc.scalar.activation(sd[:], var[:], ACTF.Sqrt)
            rstd32 = pc.tile([1, 512], F32, tag="rstd32", name="rstd32")
            nc.vector.reciprocal(rstd32[:], sd[:])
            rstd16 = pc.tile([1, 512], BF16, tag="rstd16c", name="rstd16c")
            with nc.allow_low_precision(reason="ln2 rstd bf16 bcast"):
                nc.vector.tensor_copy(rstd16[:], rstd32[:])
            pm_mu = pzc.tile([128, 512], F32, tag="bc_mu", name="bc_mu", bufs=1)
            nc.tensor.matmul(pm_mu[:], onesr[:], mu16[:], start=True, stop=True)
            pm_rs = pzc.tile([128, 512], F32, tag="bc_rs", name="bc_rs", bufs=1)
            nc.tensor.matmul(pm_rs[:], onesr[:], rstd16[:], start=True, stop=True)
            for kt in range(8):
                nc.vector.tensor_sub(tln[kt][:], x2T[kt][:], pm_mu[:])
                nc.vector.tensor_tensor(xn2b8[:, kt, :], tln[kt][:], pm_rs[:],
                                        op=ALU.mult)
            # rstd columns for router/transpose post-scaling
            for tt in range(4):
                psc = pzc.tile([128, 1], F32, tag="psc", name="psc", bufs=2)
                nc.tensor.matmul(psc[:], rstd32[0:1, tt * 128:(tt + 1) * 128],
                                 onesone[:], start=True, stop=True)
                nc.vector.tensor_copy(rstdcol[tt][:], psc[:])

          # --- router (fp32 off tln) + top-2 + slot positions ---
          with tc.tile_pool(name="phR_ps", bufs=1, space="PSUM") as pzr:
            rw = pc.tile([128, 8, 8], F32, tag="rw", name="rw")
            nc.sync.dma_start(rw[:],
                              blk3(router_wT_pad, 8))
            wgts = []
            for tt in range(4):
                pl = pzr.tile([128, 8], F32, tag="pl", name="pl", bufs=1)
                for kt in range(8):
                    nc.tensor.matmul(pl[:], tln[kt][:, tt * 128:(tt + 1) * 128],
                                     rw[:, kt, :], start=(kt == 0), stop=(kt == 7))
                t8 = pc.tile([128, 8], F32, tag="t8", name="t8", bufs=2)
                nc.vector.scalar_tensor_tensor(t8[:], pl[:], rstdcol[tt][:],
                                               biast[:], op0=ALU.mult, op1=ALU.add)
                p8r = pc.tile([128, 8], F32, tag="p8r", name="p8r", bufs=2)
                nc.scalar.activation(p8r[:], t8[:], ACTF.Sigmoid)
                mx = pc.tile([128, 8], F32, tag="mx", name="mx", bufs=2)
                nc.vector.max(mx[:], p8r[:])
                nc.vector.memset(mx[:, 2:8], -1.0)
                prep = pc.tile([128, 8], F32, tag="prep", name="prep", bufs=2)
                nc.vector.match_replace(out=prep[:], in_to_replace=mx[:],
                                        in_values=p8r[:], imm_value=0.0)
                wraw = pc.tile([128, 8], F32, tag="wraw", name="wraw", bufs=2)
                nc.vector.tensor_sub(wraw[:], p8r[:], prep[:])
                rsum = pc.tile([128, 1], F32, tag="rsum", name="rsum", bufs=2)
                nc.vector.reduce_sum(rsum[:], wraw[:], axis=AX.X)
                rrec = pc.tile([128, 1], F32, tag="rrec", name="rrec", bufs=2)
                nc.vector.reciprocal(rrec[:], rsum[:])
                wgt = pc.tile([128, 8], F32, tag=f"wgt{tt}", name=f"wgt{tt}", bufs=1)
                nc.vector.tensor_scalar(wgt[:], wraw[:], rrec[:], None, op0=ALU.mult)
                wgts.append(wgt)
                pw = pzr.tile([8, 128], F32, tag="pw", name="pw", bufs=1)
                nc.tensor.transpose(pw[:], wgt[:], ident[:])
                nc.vector.tensor_copy(w8all[:, tt * 128:(tt + 1) * 128], pw[:])

            # top-2 slot positions per expert
            mk = []
            for tt in range(4):
                m = pc.tile([128, 8], BF16, tag=f"mk{tt}", name=f"mk{tt}", bufs=1)
                nc.vector.tensor_scalar(m[:], wgts[tt][:], 0.0, None, op0=ALU.is_gt)
                mk.append(m)
            ptot = pzr.tile([1, 8], F32, tag="ptot", name="ptot", bufs=1)
            carry = []
            for tt in range(4):
                c_ = pc.tile([1, 8], F32, tag=f"carry{tt}", name=f"carry{tt}", bufs=1)
                if tt == 0:
                    nc.vector.memset(c_[:], 0.0)
                else:
                    nc.vector.tensor_copy(c_[:], ptot[:])
                carry.append(c_)
                nc.tensor.matmul(ptot[:], onesc[:], mk[tt][:],
                                 start=(tt == 0), stop=(tt == 3))
            for tt in range(4):
                pc_ = pzr.tile([128, 8], F32, tag="pcum", name="pcum", bufs=1)
                nc.tensor.matmul(pc_[:], trit[:], mk[tt][:], start=True, stop=True)
                pcb = pzr.tile([128, 8], F32, tag="pcb", name="pcb", bufs=1)
                cb16 = pc.tile([1, 8], BF16, tag="cb16", name="cb16", bufs=2)
                nc.vector.tensor_copy(cb16[:], carry[tt][:])
                nc.tensor.matmul(pcb[:], onesr[:], cb16[:], start=True, stop=True)
                t1 = pc.tile([128, 8], F32, tag="post1", name="post1", bufs=2)
                nc.vector.tensor_copy(t1[:], pc_[:])
                t2 = pc.tile([128, 8], F32, tag="post2", name="post2", bufs=2)
                nc.vector.tensor_tensor(t2[:], t1[:], pcb[:], op=ALU.add)
                t3 = pc.tile([128, 8], F32, tag="post3", name="post3", bufs=2)
                nc.vector.tensor_tensor(t3[:], t2[:], mk[tt][:], op=ALU.mult)
                nc.vector.tensor_scalar_add(posm[tt][:], t3[:], -1.0)
                pw2 = pzr.tile([8, 128], F32, tag="pw", name="pw", bufs=1)
                nc.tensor.transpose(pw2[:], posm[tt][:], ident[:])
                nc.vector.tensor_copy(pos8all[:, tt * 128:(tt + 1) * 128], pw2[:])

            # token-major xn2 via fp32 PE transposes of tln, rstd folded on copy
            for mh in range(8):
                for tt in range(4):
                    pwt = pzr.tile([128, 128], F32, tag="pwt", name="pwt", bufs=2)
                    nc.tensor.transpose(pwt[:], tln[mh][:, tt * 128:(tt + 1) * 128],
                                        ident[:])
                    nc.scalar.activation(
                        xn2tokP[:, tt, mh * 128:(mh + 1) * 128],
                        pwt[:], ACTF.Copy, scale=rstdcol[tt][:])
            # per-expert combine-weight rows
            for e in range(ER):
                pwb = pzr.tile([128, TQ], F32, tag="pwb", name="pwb", bufs=1)
                nc.tensor.matmul(pwb[:], rowselt[:, e * 128:(e + 1) * 128],
                                 w8all[:], start=True, stop=True)
                nc.scalar.activation(wb[e][:], pwb[:], ACTF.Copy)

        # ---------- phase D: MoE ----------
        with tc.tile_pool(name="phD", bufs=1) as pd_:
          selwh = [pd_.tile([96, 2, TQ], F8, tag=f"swh{i}", name=f"swh{i}")
                   for i in range(ER)]
          selwl = [pd_.tile([96, 2, TQ], F8, tag=f"swl{i}", name=f"swl{i}")
                   for i in range(ER)]
          dtok8 = [pd_.tile([96, 2, H], F8, tag=f"dt{i}", name=f"dt{i}")
                   for i in range(ER)]
          sush = pd_.tile([128, 16, TQ], F8, tag="sush", name="sush")
          sushl = pd_.tile([128, 16, TQ], F8, tag="sushl", name="sushl")
          # --- shared expert FFN first (weights stream earliest) ---
          with tc.tile_pool(name="phD_ps2", bufs=1, space="PSUM") as pz2:
            e = ER
            for gh in range(2):
                gsh = pstr.tile([128, 8, I // 2], F8, tag="gblk", name="gsh", bufs=2)
                nc.sync.dma_start(
                    gsh[:], blk3(gate_sh, 8)[:, :, gh * 1024:(gh + 1) * 1024])
                ush = pstr.tile([128, 8, I // 2], F8, tag="ublk", name="ush", bufs=2)
                nc.sync.dma_start(
                    ush[:], blk3(up_sh, 8)[:, :, gh * 1024:(gh + 1) * 1024])
                for b2 in range(8):
                    blk = gh * 8 + b2
                    pg = pz2.tile([128, TQ], F32, tag="pg2", name="pg2", bufs=2)
                    pu = pz2.tile([128, TQ], F32, tag="pu2", name="pu2", bufs=2)
                    for p2 in range(4):
                        nc.tensor.matmul(pg[:], gsh[:, 2 * p2:2 * p2 + 2,
                                                    b2 * 128:(b2 + 1) * 128],
                                         xn2b8[:, 2 * p2:2 * p2 + 2, :],
                                         start=(p2 == 0), stop=(p2 == 3),
                                         perf_mode=DR)
                        nc.tensor.matmul(pu[:], ush[:, 2 * p2:2 * p2 + 2,
                                                    b2 * 128:(b2 + 1) * 128],
                                         xn2b8[:, 2 * p2:2 * p2 + 2, :],
                                         start=(p2 == 0), stop=(p2 == 3),
                                         perf_mode=DR)
                    sg = pd_.tile([128, TQ], BF16, tag="sg2", name="sg2", bufs=2)
                    nc.scalar.activation(sg[:], pg[:], ACTF.Silu, scale=1.0 / 32)
                    # su_shared = silu(g) * u_raw = sg * pu / 32   (unit scale)
                    tmp = pd_.tile([128, TQ], F32, tag="shtmp", name="shtmp", bufs=2)
                    nc.vector.scalar_tensor_tensor(tmp[:], pu[:], 1.0 / 32,
                                                   sg[:], op0=ALU.mult, op1=ALU.mult)
                    nc.scalar.activation(sush[:, blk, :], tmp[:], ACTF.Copy)
                    nc.gpsimd.tensor_sub(sushl[:, blk, :], tmp[:], sush[:, blk, :])
            for half in range(2):
                dshh = pstr.tile([128, 16, 512], F8, tag="dbh", name=f"dshh{half}",
                                 bufs=2)
                nc.sync.dma_start(dshh[:],
                                  blk3(down_hi[e], 16)[:, :,
                                                       half * 512:(half + 1) * 512])
                dshl = pstr.tile([128, 16, 512], F8, tag="dbl", name=f"dshl{half}",
                                 bufs=2)
                nc.sync.dma_start(dshl[:],
                                  blk3(down_lo[e], 16)[:, :,
                                                       half * 512:(half + 1) * 512])
                for m2 in range(4):
                    mt = half * 4 + m2
                    ms = bass.ds(m2 * 128, 128)
                    psd = pz2.tile([128, TQ], F32, tag="pacc", name="psd", bufs=2)
                    for p2 in range(8):
                        nc.tensor.matmul(psd[:], dshh[:, 2 * p2:2 * p2 + 2, ms],
                                         sush[:, 2 * p2:2 * p2 + 2, :],
                                         start=(p2 == 0), stop=False, perf_mode=DR)
                    for p2 in range(8):
                        nc.tensor.matmul(psd[:], dshh[:, 2 * p2:2 * p2 + 2, ms],
                                         sushl[:, 2 * p2:2 * p2 + 2, :],
                                         start=False, stop=False, perf_mode=DR)
                    for p2 in range(8):
                        nc.tensor.matmul(psd[:], dshl[:, 2 * p2:2 * p2 + 2, ms],
                                         sush[:, 2 * p2:2 * p2 + 2, :],
                                         start=False, stop=(p2 == 7), perf_mode=DR)
                    nc.vector.scalar_tensor_tensor(x2T[mt][:], psd[:], 1.0 / 8,
                                                   x2T[mt][:],
                                                   op0=ALU.mult, op1=ALU.add)
          with tc.tile_pool(name="phD_ps", bufs=1, space="PSUM") as pzd:
            for e in range(ER):
                selgP = pd_.tile([128, 4, C], F8, tag="selgP", name="selgP", bufs=2)
                for tt in range(4):
                    eng = nc.vector if tt % 2 == 0 else nc.gpsimd
                    eng.tensor_scalar(selgP[:, tt, :], iota192[:],
                                      posm[tt][:, e:e + 1], None, op0=ALU.is_equal)
                xd8 = pd_.tile([128, 8, C], F8, tag="xd8", name="xd8", bufs=2)
                for mh in range(8):
                    pga = pzd.tile([128, C], F32, tag="pga", name="pga", bufs=2)
                    for t2_ in range(2):
                        nc.tensor.matmul(pga[:],
                                         xn2tokP[:, 2 * t2_:2 * t2_ + 2,
                                                 mh * 128:(mh + 1) * 128],
                                         selgP[:, 2 * t2_:2 * t2_ + 2, :],
                                         start=(t2_ == 0), stop=(t2_ == 1),
                                         perf_mode=DR)
                    nc.vector.tensor_copy(xd8[:, mh, :], pga[:])
                # positions row + weighted scatter selection
                pbe = pzd.tile([128, TQ], F32, tag="pbe", name="pbe", bufs=1)
                nc.tensor.matmul(pbe[:], rowselt[:, e * 128:(e + 1) * 128],
                                 pos8all[:], start=True, stop=True)
                for hf in range(2):
                    swt = pd_.tile([96, TQ], F32, tag="swt", name="swt", bufs=1)
                    nc.vector.scalar_tensor_tensor(
                        swt[:], pbe[0:96, :], iota96[hf][:],
                        wb[e][0:96, :], op0=ALU.is_equal, op1=ALU.mult)
                    nc.scalar.activation(selwh[e][:, hf, :], swt[:], ACTF.Copy)
                    nc.gpsimd.tensor_sub(selwl[e][:, hf, :], swt[:],
                                         selwh[e][:, hf, :])

                su16 = pd_.tile([128, 16, C], F8, tag="su16", name="su16", bufs=2)
                sul16 = pd_.tile([128, 16, C], F8, tag="sul16", name="sul16", bufs=2)
                for gh in range(2):
                    gblk = pstr.tile([128, 8, I // 2], F8, tag="gblk",
                                     name="gblk", bufs=2)
                    nc.sync.dma_start(
                        gblk[:], blk3(gate_wT[e], 8)[:, :, gh * 1024:(gh + 1) * 1024])
                    ublk = pstr.tile([128, 8, I // 2], F8, tag="ublk",
                                     name="ublk", bufs=2)
                    nc.sync.dma_start(
                        ublk[:], blk3(up_wT[e], 8)[:, :, gh * 1024:(gh + 1) * 1024])
                    for pb2 in range(4):
                        pg = pzd.tile([128, 2 * C], F32, tag="pg", name="pg", bufs=2)
                        pu = pzd.tile([128, 2 * C], F32, tag="pu", name="pu", bufs=2)
                        for i in range(2):
                            blk = pb2 * 2 + i
                            for p2 in range(4):
                                nc.tensor.matmul(pg[:, i * C:(i + 1) * C],
                                                 gblk[:, 2 * p2:2 * p2 + 2,
                                                      blk * 128:(blk + 1) * 128],
                                                 xd8[:, 2 * p2:2 * p2 + 2, :],
                                                 start=(p2 == 0), stop=(p2 == 3),
                                                 perf_mode=DR)
                                nc.tensor.matmul(pu[:, i * C:(i + 1) * C],
                                                 ublk[:, 2 * p2:2 * p2 + 2,
                                                      blk * 128:(blk + 1) * 128],
                                                 xd8[:, 2 * p2:2 * p2 + 2, :],
                                                 start=(p2 == 0), stop=(p2 == 3),
                                                 perf_mode=DR)
                        sg = pd_.tile([128, 2 * C], BF16, tag="sg", name="sg", bufs=2)
                        nc.scalar.activation(sg[:], pg[:], ACTF.Silu, scale=1.0 / 32)
                        sb2 = gh * 4 + pb2
                        tmp = pd_.tile([128, 2 * C], F32, tag="sutmp",
                                       name="sutmp", bufs=2)
                        nc.vector.tensor_tensor(tmp[:], sg[:], pu[:], op=ALU.mult)
                        suv = su16[:, 2 * sb2:2 * sb2 + 2, :].rearrange(
                            "p a b -> p (a b)")
                        nc.scalar.activation(suv[:], tmp[:], ACTF.Copy)
                        nc.gpsimd.tensor_sub(
                            sul16[:, 2 * sb2:2 * sb2 + 2, :].rearrange(
                                "p a b -> p (a b)"),
                            tmp[:], suv[:])
                for nh in range(2):
                    dbh = pstr.tile([128, 16, 512], F8, tag="dbh", name="dbh", bufs=2)
                    nc.sync.dma_start(
                        dbh[:],
                        blk3(down_hi[e], 16)[:, :, nh * 512:(nh + 1) * 512])
                    dbl = pstr.tile([128, 16, 512], F8, tag="dbl", name="dbl", bufs=2)
                    nc.sync.dma_start(
                        dbl[:],
                        blk3(down_lo[e], 16)[:, :, nh * 512:(nh + 1) * 512])
                    for c96 in range(2):
                        pd2 = pzd.tile([96, 512], F32, tag="pd", name="pd", bufs=1)
                        cs96 = bass.ds(c96 * 96, 96)
                        for p2 in range(8):
                            nc.tensor.matmul(pd2[:],
                                             su16[:, 2 * p2:2 * p2 + 2, cs96],
                                             dbh[:, 2 * p2:2 * p2 + 2, :],
                                             start=(p2 == 0), stop=False,
                                             perf_mode=DR)
                        for p2 in range(8):
                            nc.tensor.matmul(pd2[:],
                                             sul16[:, 2 * p2:2 * p2 + 2, cs96],
                                             dbh[:, 2 * p2:2 * p2 + 2, :],
                                             start=False, stop=False,
                                             perf_mode=DR)
                        for p2 in range(8):
                            nc.tensor.matmul(pd2[:],
                                             su16[:, 2 * p2:2 * p2 + 2, cs96],
                                             dbl[:, 2 * p2:2 * p2 + 2, :],
                                             start=False, stop=(p2 == 7),
                                             perf_mode=DR)
                        nc.scalar.activation(
                            dtok8[e][:, c96, nh * 512:(nh + 1) * 512], pd2[:],
                            ACTF.Copy, scale=1.0 / 128)

          # --- combine routed experts into the residual stream ---
          with tc.tile_pool(name="phD_ps3", bufs=1, space="PSUM") as pz3:
            for mt in range(8):
                pacc = pz3.tile([128, TQ], F32, tag="pacc3", name="pacc3", bufs=2)
                for e2 in range(ER):
                    nc.tensor.matmul(pacc[:],
                                     dtok8[e2][:, :, mt * 128:(mt + 1) * 128],
                                     selwh[e2][:],
                                     start=(e2 == 0), stop=False, perf_mode=DR)
                    nc.tensor.matmul(pacc[:],
                                     dtok8[e2][:, :, mt * 128:(mt + 1) * 128],
                                     selwl[e2][:],
                                     start=False, stop=(e2 == ER - 1),
                                     perf_mode=DR)
                nc.vector.scalar_tensor_tensor(x2T[mt][:], pacc[:], 1.0 / 8,
                                               x2T[mt][:], op0=ALU.mult, op1=ALU.add)
                nc.sync.dma_start(outT[mt * 128:(mt + 1) * 128, :], x2T[mt][:])

        pstr.release()

    return nc


_CACHED = {}


def kernel(**inputs):
    w, cores = host_prep(inputs)
    if 'nc' not in _CACHED:
        _CACHED['nc'] = build()
    nc = _CACHED['nc']
    in_maps = []
    for c in range(8):
        m = dict(w)
        m.update(cores[c])
        in_maps.append(m)
    res = run_bass_kernel_spmd(nc, in_maps, list(range(8)), trace=False)
    out = np.zeros((B, T, H), np.float32)
    for c in range(8):
        b, qh = c // 2, c % 2
        out[b, qh * TQ:(qh + 1) * TQ, :] = res.results[c]['outT'].T
    return out
